# revision 8
# baseline (speedup 1.0000x reference)
"""Data-parallel 3x3 conv (NHWC 16x112x112x64, OHWI 64x3x3x64, pad=1, stride=1)
on 8 TRN2 NeuronCores via Bass/Tile.

v4 strategy (per core, 2 images) -- all-dense matmuls + fused input transpose:
  - Host pre-casts x to bf16 and packs weights; device writes bf16 y (host
    upcasts). Error budget 2e-2 >> bf16 ~3e-3.
  - Host uploads x padded to 128 rows per image (rows 1..112 = data, rows
    0/113..127 = zeros). Input is transposed DIRECTLY from DRAM into
    T2[(pos2,c), j] via xbar DMA-transpose (sync queue) in full 128x128
    tiles: source AP = 128 DRAM rows (stride W*C) x nb*128 contiguous
    (col,chan). No T1, no staging hop, no pad memsets, no garbage rows.
  - T2s = partition-swapped/block-shifted copy of T2 (bulk SBUF->SBUF DMA,
    gpsimd queue, band-pipelined): T2s[0:64,j]=T2[64:128,j-128] (O of prev
    col pair), T2s[64:128,j]=T2[0:64,j+128] (E of next). This makes the
    cross-block taps a dense K=128 matmul: dense N=512 MMs sustain 216ns
    (2.4GHz) while K=64 tile_position pairs cap at ~1.6GHz.
  - Conv per 512-position chunk = 6 dense 128x128x512 matmuls in one PSUM
    bank: for dy in 0..2: mid (rhs=T2) + cross (rhs=T2s), rhs offset dy-1.
  - DMA jobs on one HWDGE queue serialize (small jobs halve descriptor
    throughput: 26 vs 14 ns/desc), so input transpose bands alternate
    between the sync and scalar queues; the out path alternates in the
    opposite phase. T2s copies + weight load ride the gpsimd SWDGE queue.
  - Vector evacuates PSUM f32 -> T3 bf16; output bands (16,16,16,4,4 blocks
    after chunks 3,7,11,12,13): xbar transpose T3 -> T4[pr,..], then DMA
    T4 -> bf16 NHWC DRAM.
"""
import sys

sys.path.insert(0, "/opt/trn_rl_repo")

import ml_dtypes
import numpy as np

import concourse.bass as bass
import concourse.tile as tile
from concourse import bacc, mybir
from concourse.bass_utils import run_bass_kernel_spmd

# Problem geometry (hardcoded per spec)
N, H, W, C = 16, 112, 112, 64
NCORES = 8
NPER = N // NCORES          # images per core
BLK = 128                   # free elems per column-pair block (2 cols x 64 ch)
NB = W // 2                 # 56 column-pair blocks per image
FPI = NB * BLK              # 7168 free elems per image in T2/T3
ZW = 256                    # zero gap width in T2/T2s around each image
F_IMG = (ZW, ZW + FPI + ZW)           # T2 start offset of each image's data
T2_LEN = ZW + FPI + ZW + FPI + ZW     # 15104
CHUNK = 512                 # positions per psum chunk (4 blocks)
CHUNKS_IMG = FPI // CHUNK   # 14

IN_BANDS = [(i * 8, i * 8 + 8) for i in range(7)]      # transpose calls
T2S_BANDS = [(0, 7), (7, 15), (15, 31), (31, 47), (47, 56)]
OUT_BANDS = [(3, 0, 16), (7, 16, 32), (11, 32, 48), (12, 48, 52), (13, 52, 56)]

f16 = mybir.dt.bfloat16
f32 = mybir.dt.float32


def _conv_kernel(tc, x_ap, w_ap, y_ap):
    nc = tc.nc
    with tc.tile_pool(name="wp", bufs=1) as wp, \
         tc.tile_pool(name="big", bufs=1) as big, \
         tc.tile_pool(name="ps", bufs=8, space="PSUM") as psp:

        wt = wp.tile([128, 768], f16)   # [3 dy x (mid 128 | cross 128)]
        nc.gpsimd.dma_start(wt[:], w_ap)

        T2 = big.tile([128, T2_LEN], f16)
        T2s = big.tile([128, T2_LEN], f16)
        T3 = big.tile([128, NPER * FPI], f16)
        T4 = big.tile([128, NPER * FPI], f16)
        T2v3 = T2[:].rearrange("p (a b) -> p a b", b=BLK)
        T4v3 = T4[:].rearrange("p (a b) -> p a b", b=BLK)

        # gap regions between/around images (read by +-1 taps and by the
        # T2s copies at image borders)
        for t in (T2, T2s):
            nc.vector.memset(t[:, 0:ZW], 0)
            nc.vector.memset(t[:, ZW + FPI: ZW + FPI + ZW], 0)
            nc.vector.memset(t[:, T2_LEN - ZW:], 0)

        xt = x_ap.tensor
        yt = y_ap.tensor
        s_row = W * C                     # DRAM row stride (elements)
        sx_img = 128 * W * C              # padded-x image stride
        s_img = H * W * C                 # y image stride

        # input: DRAM -> T2 fused xbar transposes, full 128-row tiles from
        # the host-padded x, bands alternating sync/scalar queues
        xq = [nc.sync, nc.scalar]
        xi = 0
        for img in range(NPER):
            a0 = F_IMG[img] // BLK
            for b0, b1 in IN_BANDS:
                dram = bass.AP(xt, img * sx_img + b0 * BLK,
                               [[s_row, 128], [1, (b1 - b0) * BLK]])
                xq[0].dma_start(T2v3[:, a0 + b0: a0 + b1, :], dram,
                                transpose=True)
                xi += 1

        # T2s copies (gpsimd queue), band-pipelined behind the transposes.
        # T2s block c reads T2 blocks c-1 / c+1; image borders read gaps.
        for img in range(NPER):
            base = F_IMG[img]
            for c0, c1 in T2S_BANDS:
                a, b = base + c0 * BLK, base + c1 * BLK
                nc.gpsimd.dma_start(T2s[0:64, a: b], T2[64:128, a - BLK: b - BLK])
                nc.gpsimd.dma_start(T2s[64:128, a: b], T2[0:64, a + BLK: b + BLK])

        # ---- compute: 6 dense matmuls + vector evac per chunk; output bands
        # per OUT_BANDS (scalar xpose; out-DMA alternates sync/scalar)
        T2v = T2[:]
        T2sv = T2s[:]
        out_q = [nc.sync, nc.scalar]

        def emit_chunk(img, k):
            base = F_IMG[img] + k * CHUNK
            f3 = img * FPI + k * CHUNK
            ps = psp.tile([128, CHUNK], f32)
            for dy in range(3):
                off = base + dy - 1
                m = 256 * dy
                nc.tensor.matmul(ps[:, :], wt[:, m: m + 128],
                                 T2v[:, off: off + CHUNK],
                                 start=(dy == 0), stop=False,
                                 skip_group_check=True)
                nc.tensor.matmul(ps[:, :], wt[:, m + 128: m + 256],
                                 T2sv[:, off: off + CHUNK],
                                 start=False, stop=(dy == 2),
                                 skip_group_check=True)
            nc.vector.tensor_scalar_add(T3[:, f3: f3 + CHUNK], ps[:], 0.0)

        def emit_out_band(img, b0, b1, qi):
            nb = b1 - b0
            a0 = img * (FPI // BLK) + b0
            out_q[qi % 2].dma_start(
                T4v3[:, a0: a0 + nb, :],
                T3[:, img * FPI + b0 * BLK: img * FPI + b1 * BLK],
                transpose=True)
            dram = bass.AP(yt, img * s_img + b0 * BLK,
                           [[s_row, H], [1, nb * BLK]])
            out_q[(qi + 1) % 2].dma_start(
                dram, T4[1:113, img * FPI + b0 * BLK: img * FPI + b1 * BLK])

        qi = 0
        for img in range(NPER):
            bi = 0
            for k in range(CHUNKS_IMG):
                emit_chunk(img, k)
                while bi < len(OUT_BANDS) and OUT_BANDS[bi][0] == k:
                    _, b0, b1 = OUT_BANDS[bi]
                    emit_out_band(img, b0, b1, qi)
                    qi += 1
                    bi += 1


_CACHE = {}


def _build():
    if "nc" in _CACHE:
        return _CACHE["nc"]
    nc = bacc.Bacc("TRN2", target_bir_lowering=False, debug=False,
                   num_devices=NCORES)
    x_d = nc.dram_tensor("x", [NPER * 128 * W * C], f16, kind="ExternalInput").ap()
    w_d = nc.dram_tensor("w", [128, 768], f16, kind="ExternalInput").ap()
    y_d = nc.dram_tensor("y", [NPER * H * W * C], f16, kind="ExternalOutput").ap()
    with tile.TileContext(nc) as tc:
        _conv_kernel(tc, x_d, w_d, y_d)
    nc.compile()
    _CACHE["nc"] = nc
    return nc


def _pack_weights(kernels):
    # kernels: (C_OUT=64, 3, 3, C_IN=64) f32, OHWI. wt[ci, dy, dx, co].
    wt = kernels.transpose(3, 1, 2, 0).astype(ml_dtypes.bfloat16)
    wpk = np.zeros((128, 768), ml_dtypes.bfloat16)
    for dy in range(3):
        m = 256 * dy
        # mid: rhs = T2 (E=col 2b, O=col 2b+1); M = [even out | odd out]
        wpk[0:64, m: m + 64] = wt[:, dy, 1]          # E -> even (dx=0)
        wpk[0:64, m + 64: m + 128] = wt[:, dy, 0]    # E -> odd  (dx=-1)
        wpk[64:128, m: m + 64] = wt[:, dy, 2]        # O -> even (dx=+1)
        wpk[64:128, m + 64: m + 128] = wt[:, dy, 1]  # O -> odd  (dx=0)
        # cross: rhs = T2s (O(b-1) | E(b+1)); block-diagonal
        c = m + 128
        wpk[0:64, c: c + 64] = wt[:, dy, 0]          # O(b-1) -> even (dx=-1)
        wpk[64:128, c + 64: c + 128] = wt[:, dy, 2]  # E(b+1) -> odd  (dx=+1)
    return wpk


def kernel(x, kernels, mode=None, _trace=False, **_):
    x = np.ascontiguousarray(np.asarray(x, dtype=np.float32))
    # pad each image to 128 rows: row 0 and rows 113..127 zero (conv pad
    # rows + clean xbar tiles)
    xb = np.zeros((N, 128, W, C), dtype=ml_dtypes.bfloat16)
    xb[:, 1:113] = x.astype(ml_dtypes.bfloat16)
    wpk = _pack_weights(np.asarray(kernels, dtype=np.float32))
    nc = _build()
    in_maps = [{"x": xb[i * NPER:(i + 1) * NPER].reshape(-1), "w": wpk}
               for i in range(NCORES)]
    res = run_bass_kernel_spmd(nc, in_maps, core_ids=list(range(NCORES)),
                               trace=_trace)
    out = np.concatenate(
        [np.asarray(res.results[i]["y"]).reshape(NPER, H, W, C)
         for i in range(NCORES)], axis=0)
    if _trace:
        kernel.last_result = res
    return out.astype(np.float32)


# revision 9
# speedup vs baseline: 1.5105x; 1.5105x over previous
"""Data-parallel 3x3 conv (NHWC 16x112x112x64, OHWI 64x3x3x64, pad=1, stride=1)
on 8 TRN2 NeuronCores via Bass/Tile.

v4 strategy (per core, 2 images) -- all-dense matmuls + fused input transpose:
  - Host pre-casts x to bf16 and packs weights; device writes bf16 y (host
    upcasts). Error budget 2e-2 >> bf16 ~3e-3.
  - Host uploads x padded to 128 rows per image (rows 1..112 = data, rows
    0/113..127 = zeros). Input is transposed DIRECTLY from DRAM into
    T2[(pos2,c), j] via xbar DMA-transpose (sync queue) in full 128x128
    tiles: source AP = 128 DRAM rows (stride W*C) x nb*128 contiguous
    (col,chan). No T1, no staging hop, no pad memsets, no garbage rows.
  - T2s = partition-swapped/block-shifted copy of T2 (bulk SBUF->SBUF DMA,
    gpsimd queue, band-pipelined): T2s[0:64,j]=T2[64:128,j-128] (O of prev
    col pair), T2s[64:128,j]=T2[0:64,j+128] (E of next). This makes the
    cross-block taps a dense K=128 matmul: dense N=512 MMs sustain 216ns
    (2.4GHz) while K=64 tile_position pairs cap at ~1.6GHz.
  - Conv per 512-position chunk = 6 dense 128x128x512 matmuls in one PSUM
    bank: for dy in 0..2: mid (rhs=T2) + cross (rhs=T2s), rhs offset dy-1.
  - DMA jobs on one HWDGE queue serialize (small jobs halve descriptor
    throughput: 26 vs 14 ns/desc), so input transpose bands alternate
    between the sync and scalar queues; the out path alternates in the
    opposite phase. T2s copies + weight load ride the gpsimd SWDGE queue.
  - Vector evacuates PSUM f32 -> T3 bf16; output bands (16,16,16,4,4 blocks
    after chunks 3,7,11,12,13): xbar transpose T3 -> T4[pr,..], then DMA
    T4 -> bf16 NHWC DRAM.
"""
import sys

sys.path.insert(0, "/opt/trn_rl_repo")

import ml_dtypes
import numpy as np

import concourse.bass as bass
import concourse.tile as tile
from concourse import bacc, mybir
from concourse.bass_utils import run_bass_kernel_spmd

# Problem geometry (hardcoded per spec)
N, H, W, C = 16, 112, 112, 64
NCORES = 8
NPER = N // NCORES          # images per core
BLK = 128                   # free elems per column-pair block (2 cols x 64 ch)
NB = W // 2                 # 56 column-pair blocks per image
FPI = NB * BLK              # 7168 free elems per image in T2/T3
ZW = 256                    # zero gap width in T2/T2s around each image
F_IMG = (ZW, ZW + FPI + ZW)           # T2 start offset of each image's data
T2_LEN = ZW + FPI + ZW + FPI + ZW     # 15104
CHUNK = 512                 # positions per psum chunk (4 blocks)
CHUNKS_IMG = FPI // CHUNK   # 14

IN_BANDS = [(0, 28), (28, 56)]                         # transpose calls
T2S_BANDS = [(0, 27), (27, 56)]
OUT_BANDS = [(7, 0, 32), (12, 32, 52), (13, 52, 56)]
NWARM = 9                   # PE warm-up matmuls during lead-in

f16 = mybir.dt.bfloat16
f32 = mybir.dt.float32


def _conv_kernel(tc, x_ap, w_ap, y_ap):
    nc = tc.nc
    with tc.tile_pool(name="wp", bufs=1) as wp, \
         tc.tile_pool(name="big", bufs=1) as big, \
         tc.tile_pool(name="ps", bufs=7, space="PSUM") as psp, \
         tc.tile_pool(name="pw", bufs=1, space="PSUM") as pwp:

        wt = wp.tile([128, 768], f16)   # [3 dy x (mid 128 | cross 128)]
        nc.gpsimd.dma_start(wt[:], w_ap)

        T2 = big.tile([128, T2_LEN], f16)
        T2s = big.tile([128, T2_LEN], f16)
        T3 = big.tile([128, NPER * FPI], f16)
        T4 = big.tile([128, NPER * FPI], f16)
        warm = big.tile([128, CHUNK], f16)
        T2v3 = T2[:].rearrange("p (a b) -> p a b", b=BLK)
        T4v3 = T4[:].rearrange("p (a b) -> p a b", b=BLK)

        # gap regions between/around images (read by +-1 taps and by the
        # T2s copies at image borders)
        for t in (T2, T2s):
            nc.vector.memset(t[:, 0:ZW], 0)
            nc.vector.memset(t[:, ZW + FPI: ZW + FPI + ZW], 0)
            nc.vector.memset(t[:, T2_LEN - ZW:], 0)
        nc.vector.memset(warm[:], 0)

        xt = x_ap.tensor
        yt = y_ap.tensor
        s_row = W * C                     # DRAM row stride (elements)
        sx_img = 128 * W * C              # padded-x image stride
        s_img = H * W * C                 # y image stride

        # PE warm-up: accumulate ~3.4us of PE busy during the input lead-in
        # so the HAM clock gate flips to 2.4GHz before real chunks start
        pw = pwp.tile([128, CHUNK], f32)
        for _ in range(NWARM):
            nc.tensor.matmul(pw[:, :], wt[:, 0:128], warm[:],
                             start=True, stop=True, skip_group_check=True)

        # input: DRAM -> T2 fused xbar transposes, full 128-row tiles from
        # the host-padded x. All on the sync queue: transposes into one
        # tensor from two queues race; and bigger jobs double descriptor
        # throughput (14 vs 26 ns/desc).
        for img in range(NPER):
            a0 = F_IMG[img] // BLK
            for b0, b1 in IN_BANDS:
                dram = bass.AP(xt, img * sx_img + b0 * BLK,
                               [[s_row, 128], [1, (b1 - b0) * BLK]])
                nc.sync.dma_start(T2v3[:, a0 + b0: a0 + b1, :], dram,
                                  transpose=True)

        # T2s copies (gpsimd queue), band-pipelined behind the transposes.
        # T2s block c reads T2 blocks c-1 / c+1; image borders read gaps.
        for img in range(NPER):
            base = F_IMG[img]
            for c0, c1 in T2S_BANDS:
                a, b = base + c0 * BLK, base + c1 * BLK
                nc.gpsimd.dma_start(T2s[0:64, a: b], T2[64:128, a - BLK: b - BLK])
                nc.gpsimd.dma_start(T2s[64:128, a: b], T2[0:64, a + BLK: b + BLK])

        # ---- compute: 6 dense matmuls + vector evac per chunk; output bands
        # per OUT_BANDS (scalar xpose; out-DMA alternates sync/scalar)
        T2v = T2[:]
        T2sv = T2s[:]

        def emit_chunk(img, k):
            base = F_IMG[img] + k * CHUNK
            f3 = img * FPI + k * CHUNK
            ps = psp.tile([128, CHUNK], f32)
            for dy in range(3):
                off = base + dy - 1
                m = 256 * dy
                nc.tensor.matmul(ps[:, :], wt[:, m: m + 128],
                                 T2v[:, off: off + CHUNK],
                                 start=(dy == 0), stop=False,
                                 skip_group_check=True)
                nc.tensor.matmul(ps[:, :], wt[:, m + 128: m + 256],
                                 T2sv[:, off: off + CHUNK],
                                 start=False, stop=(dy == 2),
                                 skip_group_check=True)
            nc.vector.tensor_scalar_add(T3[:, f3: f3 + CHUNK], ps[:], 0.0)

        def emit_out_band(img, b0, b1, qi):
            nb = b1 - b0
            a0 = img * (FPI // BLK) + b0
            nc.scalar.dma_start(
                T4v3[:, a0: a0 + nb, :],
                T3[:, img * FPI + b0 * BLK: img * FPI + b1 * BLK],
                transpose=True)
            dram = bass.AP(yt, img * s_img + b0 * BLK,
                           [[s_row, H], [1, nb * BLK]])
            nc.sync.dma_start(
                dram, T4[1:113, img * FPI + b0 * BLK: img * FPI + b1 * BLK])

        qi = 0
        for img in range(NPER):
            bi = 0
            for k in range(CHUNKS_IMG):
                emit_chunk(img, k)
                while bi < len(OUT_BANDS) and OUT_BANDS[bi][0] == k:
                    _, b0, b1 = OUT_BANDS[bi]
                    emit_out_band(img, b0, b1, qi)
                    qi += 1
                    bi += 1


_CACHE = {}


def _build():
    if "nc" in _CACHE:
        return _CACHE["nc"]
    nc = bacc.Bacc("TRN2", target_bir_lowering=False, debug=False,
                   num_devices=NCORES)
    x_d = nc.dram_tensor("x", [NPER * 128 * W * C], f16, kind="ExternalInput").ap()
    w_d = nc.dram_tensor("w", [128, 768], f16, kind="ExternalInput").ap()
    y_d = nc.dram_tensor("y", [NPER * H * W * C], f16, kind="ExternalOutput").ap()
    with tile.TileContext(nc) as tc:
        _conv_kernel(tc, x_d, w_d, y_d)
    nc.compile()
    _CACHE["nc"] = nc
    return nc


def _pack_weights(kernels):
    # kernels: (C_OUT=64, 3, 3, C_IN=64) f32, OHWI. wt[ci, dy, dx, co].
    wt = kernels.transpose(3, 1, 2, 0).astype(ml_dtypes.bfloat16)
    wpk = np.zeros((128, 768), ml_dtypes.bfloat16)
    for dy in range(3):
        m = 256 * dy
        # mid: rhs = T2 (E=col 2b, O=col 2b+1); M = [even out | odd out]
        wpk[0:64, m: m + 64] = wt[:, dy, 1]          # E -> even (dx=0)
        wpk[0:64, m + 64: m + 128] = wt[:, dy, 0]    # E -> odd  (dx=-1)
        wpk[64:128, m: m + 64] = wt[:, dy, 2]        # O -> even (dx=+1)
        wpk[64:128, m + 64: m + 128] = wt[:, dy, 1]  # O -> odd  (dx=0)
        # cross: rhs = T2s (O(b-1) | E(b+1)); block-diagonal
        c = m + 128
        wpk[0:64, c: c + 64] = wt[:, dy, 0]          # O(b-1) -> even (dx=-1)
        wpk[64:128, c + 64: c + 128] = wt[:, dy, 2]  # E(b+1) -> odd  (dx=+1)
    return wpk


def kernel(x, kernels, mode=None, _trace=False, **_):
    x = np.ascontiguousarray(np.asarray(x, dtype=np.float32))
    # pad each image to 128 rows: row 0 and rows 113..127 zero (conv pad
    # rows + clean xbar tiles)
    xb = np.zeros((N, 128, W, C), dtype=ml_dtypes.bfloat16)
    xb[:, 1:113] = x.astype(ml_dtypes.bfloat16)
    wpk = _pack_weights(np.asarray(kernels, dtype=np.float32))
    nc = _build()
    in_maps = [{"x": xb[i * NPER:(i + 1) * NPER].reshape(-1), "w": wpk}
               for i in range(NCORES)]
    res = run_bass_kernel_spmd(nc, in_maps, core_ids=list(range(NCORES)),
                               trace=_trace)
    out = np.concatenate(
        [np.asarray(res.results[i]["y"]).reshape(NPER, H, W, C)
         for i in range(NCORES)], axis=0)
    if _trace:
        kernel.last_result = res
    return out.astype(np.float32)


# revision 10
# speedup vs baseline: 1.5570x; 1.0308x over previous
"""Data-parallel 3x3 conv (NHWC 16x112x112x64, OHWI 64x3x3x64, pad=1, stride=1)
on 8 TRN2 NeuronCores via Bass/Tile.

v7 strategy (per core, 2 images) -- dense matmuls, fused input transpose,
per-image queue parallelism:
  - Host pre-casts x to bf16, pads each image to 128 rows (rows 1..112 =
    data, rows 0/113..127 = zeros), packs weights; device writes bf16 y
    (host upcasts). Error budget 2e-2 >> bf16 ~3e-3.
  - Input is transposed DIRECTLY from DRAM into T2[img][(pos2,c), j] via
    xbar DMA-transpose in full 128x128 tiles (j = 128*b + pr). Per-image
    tensors and queues: img0 on sync, img1 on scalar -- descriptor
    generation (~1.1ns/desc, serial per engine) and ring drain then run in
    parallel. Two queues transposing into ONE tensor race (v5 corruption),
    so destinations are separate tensors.
  - T2s[img] = partition-swapped/block-shifted copy of T2[img] (bulk
    SBUF->SBUF DMA on gpsimd): T2s[0:64,j]=T2[64:128,j-128] (O of prev col
    pair), T2s[64:128,j]=T2[0:64,j+128] (E of next). Turns the cross-block
    taps into dense K=128 matmuls: dense N=512 MMs sustain 216ns (2.4GHz)
    while K=64 tile_position pairs cap at ~1.6GHz.
  - Conv per 512-position chunk = 6 dense 128x128x512 matmuls in one PSUM
    bank: for dy in 0..2: mid (rhs=T2) + cross (rhs=T2s), rhs offset dy-1.
  - 9 warm-up matmuls during the lead-in flip the PE HAM clock gate to
    2.4GHz before real chunks start.
  - Vector evacuates PSUM f32 -> T3 bf16; output bands (32,20,4 blocks
    after chunks 7,12,13): xbar transpose T3 -> T4[img][pr,..] (img0 on
    scalar, img1 on sync -- opposite phase to input), then gpsimd DMA
    T4 -> bf16 NHWC DRAM.
"""
import sys

sys.path.insert(0, "/opt/trn_rl_repo")

import ml_dtypes
import numpy as np

import concourse.bass as bass
import concourse.tile as tile
from concourse import bacc, mybir
from concourse.bass_utils import run_bass_kernel_spmd

# Problem geometry (hardcoded per spec)
N, H, W, C = 16, 112, 112, 64
NCORES = 8
NPER = N // NCORES          # images per core
BLK = 128                   # free elems per column-pair block (2 cols x 64 ch)
NB = W // 2                 # 56 column-pair blocks per image
FPI = NB * BLK              # 7168 free elems per image
ZW = 256                    # zero gap width around each image
TLEN = ZW + FPI + ZW        # per-image T2/T2s length (7680)
CHUNK = 512                 # positions per psum chunk (4 blocks)
CHUNKS_IMG = FPI // CHUNK   # 14
NWARM = 9                   # PE warm-up matmuls during lead-in

IN_BANDS = [(0, 28), (28, 56)]
T2S_BANDS = [(0, 27), (27, 56)]
OUT_BANDS = [(7, 0, 32), (12, 32, 52), (13, 52, 56)]

f16 = mybir.dt.bfloat16
f32 = mybir.dt.float32


def _conv_kernel(tc, x_ap, w_ap, y_ap):
    nc = tc.nc
    with tc.tile_pool(name="wp", bufs=1) as wp, \
         tc.tile_pool(name="big", bufs=1) as big, \
         tc.tile_pool(name="ps", bufs=7, space="PSUM") as psp, \
         tc.tile_pool(name="pw", bufs=1, space="PSUM") as pwp:

        wt = wp.tile([128, 768], f16)   # [3 dy x (mid 128 | cross 128)]
        nc.gpsimd.dma_start(wt[:], w_ap)

        T2a = big.tile([128, TLEN], f16)
        T2b = big.tile([128, TLEN], f16)
        T2sa = big.tile([128, TLEN], f16)
        T2sb = big.tile([128, TLEN], f16)
        T3 = big.tile([128, NPER * FPI], f16)
        T4a = big.tile([128, FPI], f16)
        T4b = big.tile([128, FPI], f16)
        warm = big.tile([128, CHUNK], f16)
        T2 = (T2a, T2b)
        T2s = (T2sa, T2sb)
        T4 = (T4a, T4b)

        # gap regions (read by +-1 taps and by T2s copies at image borders)
        for t in (T2a, T2b, T2sa, T2sb):
            nc.vector.memset(t[:, 0:ZW], 0)
            nc.vector.memset(t[:, TLEN - ZW:], 0)
        nc.vector.memset(warm[:], 0)

        xt = x_ap.tensor
        yt = y_ap.tensor
        s_row = W * C                     # DRAM row stride (elements)
        sx_img = 128 * W * C              # padded-x image stride
        s_img = H * W * C                 # y image stride

        # PE warm-up: accumulate ~3.4us of PE busy during the input lead-in
        # so the HAM clock gate flips to 2.4GHz before real chunks start
        pw = pwp.tile([128, CHUNK], f32)
        for _ in range(NWARM):
            nc.tensor.matmul(pw[:, :], wt[:, 0:128], warm[:],
                             start=True, stop=True, skip_group_check=True)

        # input: DRAM -> T2[img] fused xbar transposes, full 128-row tiles
        # from the host-padded x; img0 on sync, img1 on scalar (parallel
        # desc-gen + rings, separate dst tensors)
        in_q = (nc.sync, nc.scalar)
        for img in range(NPER):
            t2v3 = T2[img][:].rearrange("p (a b) -> p a b", b=BLK)
            a0 = ZW // BLK
            for b0, b1 in IN_BANDS:
                dram = bass.AP(xt, img * sx_img + b0 * BLK,
                               [[s_row, 128], [1, (b1 - b0) * BLK]])
                in_q[img].dma_start(t2v3[:, a0 + b0: a0 + b1, :], dram,
                                    transpose=True)

        # T2s copies (gpsimd), band-pipelined behind the transposes
        for img in range(NPER):
            for c0, c1 in T2S_BANDS:
                a, b = ZW + c0 * BLK, ZW + c1 * BLK
                nc.gpsimd.dma_start(T2s[img][0:64, a: b],
                                    T2[img][64:128, a - BLK: b - BLK])
                nc.gpsimd.dma_start(T2s[img][64:128, a: b],
                                    T2[img][0:64, a + BLK: b + BLK])

        # ---- compute: 6 dense matmuls + vector evac per chunk; output
        # bands per OUT_BANDS (out-xpose: img0 scalar / img1 sync; out-DMA
        # on gpsimd)
        out_q = (nc.scalar, nc.sync)

        def emit_chunk(img, k):
            base = ZW + k * CHUNK
            f3 = img * FPI + k * CHUNK
            t2v, t2sv = T2[img][:], T2s[img][:]
            ps = psp.tile([128, CHUNK], f32)
            for dy in range(3):
                off = base + dy - 1
                m = 256 * dy
                nc.tensor.matmul(ps[:, :], wt[:, m: m + 128],
                                 t2v[:, off: off + CHUNK],
                                 start=(dy == 0), stop=False,
                                 skip_group_check=True)
                nc.tensor.matmul(ps[:, :], wt[:, m + 128: m + 256],
                                 t2sv[:, off: off + CHUNK],
                                 start=False, stop=(dy == 2),
                                 skip_group_check=True)
            nc.vector.tensor_scalar_add(T3[:, f3: f3 + CHUNK], ps[:], 0.0)

        def emit_out_band(img, b0, b1):
            nb = b1 - b0
            t4v3 = T4[img][:].rearrange("p (a b) -> p a b", b=BLK)
            out_q[img].dma_start(
                t4v3[:, b0: b0 + nb, :],
                T3[:, img * FPI + b0 * BLK: img * FPI + b1 * BLK],
                transpose=True)
            dram = bass.AP(yt, img * s_img + b0 * BLK,
                           [[s_row, H], [1, nb * BLK]])
            nc.gpsimd.dma_start(
                dram, T4[img][1:113, b0 * BLK: b1 * BLK])

        for img in range(NPER):
            bi = 0
            for k in range(CHUNKS_IMG):
                emit_chunk(img, k)
                while bi < len(OUT_BANDS) and OUT_BANDS[bi][0] == k:
                    _, b0, b1 = OUT_BANDS[bi]
                    emit_out_band(img, b0, b1)
                    bi += 1


_CACHE = {}


def _build():
    if "nc" in _CACHE:
        return _CACHE["nc"]
    nc = bacc.Bacc("TRN2", target_bir_lowering=False, debug=False,
                   num_devices=NCORES)
    x_d = nc.dram_tensor("x", [NPER * 128 * W * C], f16, kind="ExternalInput").ap()
    w_d = nc.dram_tensor("w", [128, 768], f16, kind="ExternalInput").ap()
    y_d = nc.dram_tensor("y", [NPER * H * W * C], f16, kind="ExternalOutput").ap()
    with tile.TileContext(nc) as tc:
        _conv_kernel(tc, x_d, w_d, y_d)
    nc.compile()
    _CACHE["nc"] = nc
    return nc


def _pack_weights(kernels):
    # kernels: (C_OUT=64, 3, 3, C_IN=64) f32, OHWI. wt[ci, dy, dx, co].
    wt = kernels.transpose(3, 1, 2, 0).astype(ml_dtypes.bfloat16)
    wpk = np.zeros((128, 768), ml_dtypes.bfloat16)
    for dy in range(3):
        m = 256 * dy
        # mid: rhs = T2 (E=col 2b, O=col 2b+1); M = [even out | odd out]
        wpk[0:64, m: m + 64] = wt[:, dy, 1]          # E -> even (dx=0)
        wpk[0:64, m + 64: m + 128] = wt[:, dy, 0]    # E -> odd  (dx=-1)
        wpk[64:128, m: m + 64] = wt[:, dy, 2]        # O -> even (dx=+1)
        wpk[64:128, m + 64: m + 128] = wt[:, dy, 1]  # O -> odd  (dx=0)
        # cross: rhs = T2s (O(b-1) | E(b+1)); block-diagonal
        c = m + 128
        wpk[0:64, c: c + 64] = wt[:, dy, 0]          # O(b-1) -> even (dx=-1)
        wpk[64:128, c + 64: c + 128] = wt[:, dy, 2]  # E(b+1) -> odd  (dx=+1)
    return wpk


def kernel(x, kernels, mode=None, _trace=False, **_):
    x = np.ascontiguousarray(np.asarray(x, dtype=np.float32))
    # pad each image to 128 rows: row 0 and rows 113..127 zero (conv pad
    # rows + clean xbar tiles)
    xb = np.zeros((N, 128, W, C), dtype=ml_dtypes.bfloat16)
    xb[:, 1:113] = x.astype(ml_dtypes.bfloat16)
    wpk = _pack_weights(np.asarray(kernels, dtype=np.float32))
    nc = _build()
    in_maps = [{"x": xb[i * NPER:(i + 1) * NPER].reshape(-1), "w": wpk}
               for i in range(NCORES)]
    res = run_bass_kernel_spmd(nc, in_maps, core_ids=list(range(NCORES)),
                               trace=_trace)
    out = np.concatenate(
        [np.asarray(res.results[i]["y"]).reshape(NPER, H, W, C)
         for i in range(NCORES)], axis=0)
    if _trace:
        kernel.last_result = res
    return out.astype(np.float32)
